# revision 4
# baseline (speedup 1.0000x reference)
"""CrossAttention Trainium2 kernel — 8-way sharded (batch x head-group).

Sharding: core i -> batch b = i//2, head-group g = i%2 (8 of 16 heads).
Each core computes its batch's attention for its 8 heads and a partial
output projection (row-parallel Wo); host sums the two partials per batch.

All matmuls run in float32r (full PE rate). Scores are computed transposed
(scoresT[kv, q]) so exp(scoresT) feeds the AV matmul directly as the
stationary operand with no transpose. A ones-column appended to V yields
softmax denominators for free; normalization is applied to ctxT before the
output projection. scale_factor/sqrt(HD) and bq are folded into Wq/bq on
the host; bv/bo enter the output linearly and are added on the host.
"""
import math
import numpy as np

import concourse.bass as bass
import concourse.mybir as mybir
import concourse.tile as tile
from concourse import bacc
from concourse.bass_utils import run_bass_kernel_spmd

F32 = mybir.dt.float32
FR = mybir.dt.float32r

B, LQ, LKV, D, H, HD = 4, 1024, 2048, 1024, 16, 64
DG = 512          # head-group width (8 heads x 64)
NCORES = 8

_NC_CACHE = None


def _build_nc():
    nc = bacc.Bacc(None, target_bir_lowering=False)

    xqT = nc.declare_dram_parameter("xqT", [D, LQ], FR, isOutput=False)
    xkvT = nc.declare_dram_parameter("xkvT", [D, LKV], FR, isOutput=False)
    wqT = nc.declare_dram_parameter("wqT", [D, DG], FR, isOutput=False)
    wkT = nc.declare_dram_parameter("wkT", [D, DG], FR, isOutput=False)
    wvT = nc.declare_dram_parameter("wvT", [D, DG], FR, isOutput=False)
    woT = nc.declare_dram_parameter("woT", [DG, D], FR, isOutput=False)
    bq2 = nc.declare_dram_parameter("bq2", [128, 4], F32, isOutput=False)
    bk2 = nc.declare_dram_parameter("bk2", [128, 4], F32, isOutput=False)
    ones8 = nc.declare_dram_parameter("ones8", [128, 8], FR, isOutput=False)
    out = nc.declare_dram_parameter("out", [LQ, D], F32, isOutput=True)

    # DRAM tiled views: [n, 128, cols]
    xqT_t = xqT[:].rearrange("(n p) m -> n p m", p=128)     # 8 x [128, 1024]
    xkvT_t = xkvT[:].rearrange("(n p) m -> n p m", p=128)   # 8 x [128, 2048]
    wqT_t = wqT[:].rearrange("(n p) m -> n p m", p=128)     # 8 x [128, 512]
    wkT_t = wkT[:].rearrange("(n p) m -> n p m", p=128)
    wvT_t = wvT[:].rearrange("(n p) m -> n p m", p=128)
    woT_t = woT[:].rearrange("(n p) m -> n p m", p=128)     # 4 x [128, 1024]

    Ident = mybir.ActivationFunctionType.Identity
    Exp = mybir.ActivationFunctionType.Exp

    with tile.TileContext(nc) as tc:
        with tc.tile_pool(name="pers", bufs=1) as pers, \
             tc.tile_pool(name="psp", bufs=1, space="PSUM") as psp, \
             tc.tile_pool(name="attnp", bufs=1) as attnp, \
             tc.tile_pool(name="invp", bufs=1) as invp:

            # persistent activations
            qT = [pers.tile([128, LQ], FR, tag=f"qT{i}", name=f"qT{i}") for i in range(4)]
            kT = [pers.tile([128, LKV], FR, tag=f"kT{i}", name=f"kT{i}") for i in range(4)]
            vaug = [pers.tile([128, 8 * 65], FR, tag=f"va{i}", name=f"va{i}") for i in range(16)]
            bq_sb = pers.tile([128, 4], F32, tag="bq_sb")
            bk_sb = pers.tile([128, 4], F32, tag="bk_sb")
            nc.sync.dma_start(bq_sb[:], bq2[:])
            nc.sync.dma_start(bk_sb[:], bk2[:])
            # ones column of v_aug, direct from DRAM into strided columns
            for m in range(16):
                dst = vaug[m][:].rearrange("p (h c) -> p h c", c=65)[:, :, 64]
                nc.sync.dma_start(dst, ones8[:])

            # ---------------- Q projection: qT[dout, lq] ----------------
            with tc.tile_pool(name="xqp", bufs=1) as xqp, \
                 tc.tile_pool(name="wqp", bufs=1) as wqp:
                xq_sb = []
                wq_sb = []
                for k in range(8):
                    t = xqp.tile([128, LQ], FR, tag=f"xq{k}", name=f"xq{k}")
                    nc.sync.dma_start(t[:], xqT_t[k])
                    xq_sb.append(t)
                    w = wqp.tile([128, DG], FR, tag=f"wq{k}", name=f"wq{k}")
                    nc.sync.dma_start(w[:], wqT_t[k])
                    wq_sb.append(w)
                for m in range(4):           # dout chunk (qT partition tile)
                    for n in range(2):       # lq chunk of 512
                        ps = psp.tile([128, 1024], F32, tag="sc", bufs=2, name="scps")
                        for k in range(8):
                            nc.tensor.matmul(
                                ps[:, :512],
                                wq_sb[k][:, m * 128:(m + 1) * 128],
                                xq_sb[k][:, n * 512:(n + 1) * 512],
                                start=(k == 0), stop=(k == 7),
                            )
                        nc.scalar.activation(
                            qT[m][:, n * 512:(n + 1) * 512], ps[:, :512],
                            Ident, bias=bq_sb[:, m:m + 1], scale=1.0,
                        )

            # ------------- K/V projections, kv split in halves -------------
            with tc.tile_pool(name="xkvp", bufs=1) as xkvp, \
                 tc.tile_pool(name="wkvp", bufs=1) as wkvp:
                wk_sb, wv_sb = [], []
                for k in range(8):
                    w = wkvp.tile([128, DG], FR, tag=f"wk{k}", name=f"wk{k}")
                    nc.sync.dma_start(w[:], wkT_t[k])
                    wk_sb.append(w)
                    w = wkvp.tile([128, DG], FR, tag=f"wv{k}", name=f"wv{k}")
                    nc.sync.dma_start(w[:], wvT_t[k])
                    wv_sb.append(w)
                for half in range(2):
                    cols = slice(half * 1024, (half + 1) * 1024)
                    xkv_sb = []
                    for k in range(8):
                        t = xkvp.tile([128, 1024], FR, tag=f"xkv{k}", bufs=1, name=f"xkv{k}")
                        nc.sync.dma_start(t[:], xkvT_t[k][:, cols])
                        xkv_sb.append(t)
                    # K proj -> kT[m][:, kv in this half]
                    for m in range(4):
                        for n in range(2):    # kv chunk of 512 within half
                            ps = psp.tile([128, 1024], F32, tag="sc", bufs=2, name="scps")
                            for k in range(8):
                                nc.tensor.matmul(
                                    ps[:, :512],
                                    wk_sb[k][:, m * 128:(m + 1) * 128],
                                    xkv_sb[k][:, n * 512:(n + 1) * 512],
                                    start=(k == 0), stop=(k == 7),
                                )
                            off = half * 1024 + n * 512
                            nc.scalar.activation(
                                kT[m][:, off:off + 512], ps[:, :512],
                                Ident, bias=bk_sb[:, m:m + 1], scale=1.0,
                            )
                    # V proj -> vaug[kv tile][:, interleaved head cols]
                    for m in range(8):        # kv chunk of 128 within half
                        ps = psp.tile([128, 1024], F32, tag="sc", bufs=2, name="scps")
                        for k in range(8):
                            nc.tensor.matmul(
                                ps[:, :512],
                                xkv_sb[k][:, m * 128:(m + 1) * 128],
                                wv_sb[k][:],
                                start=(k == 0), stop=(k == 7),
                            )
                        mt = half * 8 + m
                        dst = vaug[mt][:].rearrange("p (h c) -> p h c", c=65)[:, :, 0:64]
                        src = ps[:, :512].rearrange("p (h c) -> p h c", c=64)
                        nc.vector.tensor_copy(dst, src)

            # ---------------- attention, per head pair ----------------
            ctxT = [pers.tile([128, LQ], FR, tag=f"cT{i}", name=f"cT{i}") for i in range(4)]
            for p in range(4):
                av_ps = {}
                for h2 in range(2):
                    for qh in range(2):
                        av_ps[(h2, qh)] = psp.tile([65, 512], F32, tag="av", bufs=4, name="avps")
                sc_ps = {}
                for kc in range(16):
                    for h2 in range(2):
                        r = h2 * 64
                        ps = psp.tile([128, 1024], F32, tag="sc", bufs=2, name="scps")
                        sc_ps[h2] = ps
                        for qh in range(2):
                            nc.tensor.matmul(
                                ps[:, qh * 512:(qh + 1) * 512],
                                kT[p][r:r + 64, kc * 128:(kc + 1) * 128],
                                qT[p][r:r + 64, qh * 512:(qh + 1) * 512],
                                start=True, stop=True,
                                tile_position=(r, 0),
                            )
                    for h2 in range(2):
                        h = 2 * p + h2
                        at = attnp.tile([128, 1024], FR, tag="attnT", bufs=4, name="attnT")
                        nc.scalar.activation(at[:], sc_ps[h2][:], Exp)
                        for qh in range(2):
                            nc.tensor.matmul(
                                av_ps[(h2, qh)][:],
                                vaug[kc][:, h * 65:h * 65 + 65],
                                at[:, qh * 512:(qh + 1) * 512],
                                start=(kc == 0), stop=(kc == 15),
                            )
                for h2 in range(2):
                    r = h2 * 64
                    for qh in range(2):
                        ps = av_ps[(h2, qh)]
                        inv = invp.tile([1, 512], F32, tag="inv", bufs=4, name="inv")
                        nc.vector.reciprocal(inv[:], ps[64:65, :])
                        invb = invp.tile([64, 512], F32, tag="invb", bufs=2, name="invb")
                        nc.gpsimd.partition_broadcast(invb[:], inv[:])
                        nc.vector.tensor_mul(
                            ctxT[p][r:r + 64, qh * 512:(qh + 1) * 512],
                            ps[0:64, :],
                            invb[:],
                        )

            # ---------------- output projection ----------------
            with tc.tile_pool(name="wop", bufs=1) as wop, \
                 tc.tile_pool(name="outp", bufs=4) as outp:
                wo_sb = []
                for k in range(4):
                    w = wop.tile([128, D], FR, tag=f"wo{k}", name=f"wo{k}")
                    nc.sync.dma_start(w[:], woT_t[k])
                    wo_sb.append(w)
                for m in range(8):            # lq chunk of 128
                    for n in range(2):        # dout chunk of 512
                        ps = psp.tile([128, 1024], F32, tag="sc", bufs=2, name="scps")
                        for k in range(4):
                            nc.tensor.matmul(
                                ps[:, :512],
                                ctxT[k][:, m * 128:(m + 1) * 128],
                                wo_sb[k][:, n * 512:(n + 1) * 512],
                                start=(k == 0), stop=(k == 3),
                            )
                        ot = outp.tile([128, 512], F32, tag="ot", bufs=4, name="ot")
                        nc.vector.tensor_copy(ot[:], ps[:, :512])
                        nc.sync.dma_start(
                            out[m * 128:(m + 1) * 128, n * 512:(n + 1) * 512],
                            ot[:],
                        )

    nc.compile()
    return nc


def _get_nc():
    global _NC_CACHE
    if _NC_CACHE is None:
        _NC_CACHE = _build_nc()
    return _NC_CACHE


def _prep_in_maps(query_domain, key_value_domain, Wq, bq, Wk, bk, Wv, bv,
                  Wo, bo, scale_factor, beta):
    s = float(scale_factor.reshape(-1)[0]) / math.sqrt(HD)
    f32 = np.float32
    q = np.ascontiguousarray(query_domain, dtype=f32)
    kv = np.ascontiguousarray(key_value_domain, dtype=f32)
    Wq = np.asarray(Wq, dtype=f32)
    Wk = np.asarray(Wk, dtype=f32)
    Wv = np.asarray(Wv, dtype=f32)
    Wo = np.asarray(Wo, dtype=f32)
    ones8 = np.ones((128, 8), dtype=f32)
    in_maps = []
    for core in range(NCORES):
        b, g = core // 2, core % 2
        gsl = slice(g * DG, (g + 1) * DG)
        in_maps.append({
            "xqT": np.ascontiguousarray(q[b].T),
            "xkvT": np.ascontiguousarray(kv[b].T),
            "wqT": np.ascontiguousarray((Wq[gsl] * s).T),
            "wkT": np.ascontiguousarray(Wk[gsl].T),
            "wvT": np.ascontiguousarray(Wv[gsl].T),
            "woT": np.ascontiguousarray(Wo[:, gsl].T),
            "bq2": np.ascontiguousarray(
                (np.asarray(bq, f32)[gsl] * s).reshape(4, 128).T),
            "bk2": np.ascontiguousarray(
                np.asarray(bk, f32)[gsl].reshape(4, 128).T),
            "ones8": ones8,
        })
    return in_maps


def _run(inputs, trace=False, trace_kwargs=None):
    nc = _get_nc()
    in_maps = _prep_in_maps(**inputs)
    res = run_bass_kernel_spmd(
        nc, in_maps, list(range(NCORES)), trace=trace,
        trace_kwargs=trace_kwargs or {},
    )
    f32 = np.float32
    Wo = np.asarray(inputs["Wo"], dtype=f32)
    bv = np.asarray(inputs["bv"], dtype=f32)
    bo = np.asarray(inputs["bo"], dtype=f32)
    c_total = (bv @ Wo.T + bo).astype(f32)
    out = np.empty((B, LQ, D), dtype=f32)
    for b in range(B):
        out[b] = res.results[2 * b]["out"] + res.results[2 * b + 1]["out"] + c_total
    return out, res


def kernel(**inputs):
    out, _ = _run(inputs)
    return out
